# revision 4
# baseline (speedup 1.0000x reference)
"""Trainium2 Bass kernel v2 for causal multi-head attention with RoPE.

Problem: B=2, S=2048, D=2048, H=16 heads (HD=128), fp32 reference.
Sharding (8 NeuronCores): 2-way batch x 4-way heads; host sums the 4
wo-partials per batch element.

v2 changes vs v1 (368.6us baseline):
  - Multi-engine DMA issue (sync/scalar/gpsimd HW queues in parallel)
    with 512KB grouped descriptors; host pre-groups weight/x layouts.
  - HAM warmup: junk matmuls at t=0 so real matmuls run at 2.4 GHz.
  - Fine-grained causal trim on diagonal super-blocks (ranged matmuls).
  - Running-sum softmax denominator; one ones-matmul per (head, chunk).
  - RoPE rotate-half via DVE stream_shuffle (no SBUF-SBUF DMA).
  - Software-pipelined emission: attention j-tiles of chunk qc are
    injected between projection/outproj matmul chains of neighboring
    chunks so the PE never stalls on the exp (ScalarE) latency.
  - Output written via one 512KB DMA per 128-token row block.
"""

import math
from collections import deque

import numpy as np
import ml_dtypes

import concourse.bass as bass
import concourse.mybir as mybir
import concourse.tile as tile
from concourse import bacc, bass_isa, bass_utils

BF16 = ml_dtypes.bfloat16
F32 = mybir.dt.float32
BF = mybir.dt.bfloat16

B, S, D, H = 2, 2048, 2048, 16
HD = 128
NCORE = 8
HPC = 4            # heads per core
OSL = HPC * HD     # 512-wide output slice per core
NT = S // 128      # 16 token tiles
ND = D // 128      # 16 contraction tiles
NCH = 4            # 512-wide token chunks
SCALE = 1.0 / math.sqrt(HD)
# stream_shuffle permutes within each 32-partition quadrant; the host
# packs each quadrant as [16 re rows, 16 im rows] so this mask swaps them
SWAP_MASK = [(i + 16) % 32 for i in range(32)]
NJUNK = 48


def _build_program():
    nc = bacc.Bacc(
        "TRN2",
        target_bir_lowering=False,
        debug=False,
        enable_asserts=False,
        num_devices=NCORE,
    )
    # host pre-grouped layouts (see _host_prep):
    #  xtg row (qc*4+g)*128+p holds x^T tiles d=4g..4g+3 of chunk qc
    #  w*g row g*128+p holds weight tiles d=4g..4g+3
    xtg = nc.dram_tensor("xtg", [S, 2048], BF, kind="ExternalInput").ap()
    wqg = nc.dram_tensor("wqg", [OSL, 2048], BF, kind="ExternalInput").ap()
    wkg = nc.dram_tensor("wkg", [OSL, 2048], BF, kind="ExternalInput").ap()
    wvg = nc.dram_tensor("wvg", [OSL, 2048], BF, kind="ExternalInput").ap()
    woT = nc.dram_tensor("woT", [OSL, D], BF, kind="ExternalInput").ap()
    cos2 = nc.dram_tensor("cos2", [128, S], BF, kind="ExternalInput").ap()
    sin2 = nc.dram_tensor("sin2", [128, S], BF, kind="ExternalInput").ap()
    tri = nc.dram_tensor("tri", [128, 128], BF, kind="ExternalInput").ap()
    out = nc.dram_tensor("out", [S, D], BF, kind="ExternalOutput").ap()

    with tile.TileContext(nc) as tc:
        _kernel_body(tc, xtg, wqg, wkg, wvg, woT, cos2, sin2, tri, out)
    nc.compile()
    return nc


def _kernel_body(tc, xtg, wqg, wkg, wvg, woT, cos2, sin2, tri, out):
    nc = tc.nc
    Exp = mybir.ActivationFunctionType.Exp

    with (
        tc.tile_pool(name="weights", bufs=1) as wpool,
        tc.tile_pool(name="consts", bufs=1) as cpool,
        tc.tile_pool(name="qkv", bufs=1) as qkvpool,
        tc.tile_pool(name="xtp", bufs=3) as xpool,
        tc.tile_pool(name="rope", bufs=1) as rpool,
        tc.tile_pool(name="pg", bufs=3) as ppool,
        tc.tile_pool(name="accs", bufs=2) as apool,
        tc.tile_pool(name="smallsb", bufs=2) as spool,
        tc.tile_pool(name="outsb", bufs=2) as outpool,
        tc.tile_pool(name="floatps", bufs=2, space="PSUM") as floatps,
        tc.tile_pool(name="scps", bufs=2, space="PSUM") as scps,
        tc.tile_pool(name="ctxps", bufs=2, space="PSUM") as ctxps,
    ):
        wq_s = wpool.tile([128, ND, OSL], BF, tag="wq")
        wk_s = wpool.tile([128, ND, OSL], BF, tag="wk")
        wv_s = wpool.tile([128, ND, OSL], BF, tag="wv")
        wo_s = wpool.tile([128, HPC, D], BF, tag="wo")
        cos_s = cpool.tile([128, S], BF, tag="cos")
        sin_s = cpool.tile([128, S], BF, tag="sin")
        tri_s = cpool.tile([128, 128], BF, tag="tri")
        ones_s = cpool.tile([128, 1], BF, tag="ones")
        ones_r = cpool.tile([1, 128], BF, tag="onesr")
        junk_w = cpool.tile([128, 128], BF, tag="junkw")
        qt = [qkvpool.tile([128, S], BF, tag=f"qt{h}", name=f"qt{h}")
              for h in range(HPC)]
        kt = [qkvpool.tile([128, S], BF, tag=f"kt{h}", name=f"kt{h}")
              for h in range(HPC)]
        v_s = qkvpool.tile([128, NT, OSL], BF, tag="v")
        ctxT = [qkvpool.tile([128, S], BF, tag=f"ctx{h}", name=f"ctxT{h}")
                for h in range(HPC)]

        # ---- t=0: junk-MM HAM warmup + parallel DMA issue ---------------
        nc.gpsimd.memset(junk_w[:], 0.0)
        nc.gpsimd.memset(ones_s[:], 1.0)
        nc.gpsimd.memset(ones_r[:], 1.0)
        jps = floatps.tile([128, 512], F32, tag="float", name="junkps")
        for i in range(NJUNK):
            nc.tensor.matmul(jps[:, 0:128], junk_w[:], junk_w[:],
                             start=True, stop=True)

        xt_tiles = {}

        def emit_xt_dma(qc, fine=False):
            t = xpool.tile([128, ND, 512], BF, tag="xt", name=f"xt{qc}")
            xt_tiles[qc] = t
            if fine:  # first two groups tile-by-tile for faster first MMs
                for d in range(8):
                    r0 = (qc * 4 + d // 4) * 128
                    nc.sync.dma_start(
                        t[:, d, :],
                        xtg[r0:r0 + 128, (d % 4) * 512:(d % 4 + 1) * 512])
                gs = range(2, 4)
            else:
                gs = range(4)
            for g in gs:
                r0 = (qc * 4 + g) * 128
                nc.sync.dma_start(t[:, g * 4:(g + 1) * 4, :],
                                  xtg[r0:r0 + 128, :])

        emit_xt_dma(0, fine=True)
        # cos/sin ride the sync queue behind xt0: wire-serialized after the
        # phase-A-critical bytes, still landing before the first rope
        nc.sync.dma_start(cos_s[:], cos2[:])
        nc.sync.dma_start(sin_s[:], sin2[:])
        # scalar engine: wq (first two groups at single-tile granularity so
        # phase A starts sooner), then wk, then wv
        for d in range(8):
            nc.scalar.dma_start(wq_s[:, d, :],
                                wqg[(d // 4) * 128:(d // 4 + 1) * 128,
                                    (d % 4) * 512:(d % 4 + 1) * 512])
        for g in range(2, 4):
            nc.scalar.dma_start(wq_s[:, g * 4:(g + 1) * 4, :],
                                wqg[g * 128:(g + 1) * 128, :])
        for g in range(4):
            nc.scalar.dma_start(wk_s[:, g * 4:(g + 1) * 4, :],
                                wkg[g * 128:(g + 1) * 128, :])
        for g in range(4):
            nc.scalar.dma_start(wv_s[:, g * 4:(g + 1) * 4, :],
                                wvg[g * 128:(g + 1) * 128, :])
        # gpsimd engine: cos/sin (needed ~17us), tri, wo
        nc.gpsimd.dma_start(tri_s[:], tri[:])
        # cos/sin (needed ~20us), wo (~70us) and xt1 (~45us) are issued
        # later to keep early HBM bandwidth for wq/wk/xt0

        # ---- attention emission machinery -------------------------------
        pending = deque()   # (qc, h, jE) pair entries, in required order
        vdone = {}          # qc -> number of V-chain evacs emitted
        state = {}          # (qc, h) -> dict(acc=, ctx=)
        sched = {"budget": 0.0, "ratio": 5.0, "qc": 0}

        fifo = deque()  # scored-but-not-PV'd pairs (lag-2 pipeline)

        def emit_scores_part(qc, h, jE):
            st = state.setdefault((qc, h), {"acc": None, "ctx": None})
            if st["ctx"] is None:
                st["ctx"] = ctxps.tile([128, 512], F32, tag="ctx",
                                       name=f"ctx{qc}_{h}")
            pair = ppool.tile([128, 2, 512], BF, tag="pg",
                              name=f"pg{qc}_{h}_{jE}")
            sc = scps.tile([128, 2, 512], F32, tag="sc",
                           name=f"sc{qc}_{h}_{jE}")
            halves = [(0, jE), (1, jE + 1)]
            for half, j in halves:
                r = j - 4 * qc
                lo = 128 * r if r > 0 else 0
                nc.tensor.matmul(
                    sc[:, half, lo:512],
                    kt[h][:, j * 128:(j + 1) * 128],
                    qt[h][:, qc * 512 + lo:(qc + 1) * 512],
                    start=True, stop=True)
            if jE >= 4 * qc:  # diagonal pair: per-half restricted exp + mask
                for half, j in halves:
                    r = j - 4 * qc
                    lo = 128 * r
                    nc.scalar.activation(pair[:, half, lo:512],
                                         sc[:, half, lo:512], Exp,
                                         scale=SCALE)
                    nc.vector.tensor_mul(pair[:, half, lo:lo + 128],
                                         pair[:, half, lo:lo + 128],
                                         tri_s[:])
            else:
                nc.scalar.activation(pair[:], sc[:], Exp, scale=SCALE)
            return (qc, h, jE, pair, st)

        def emit_pv_part(info):
            qc, h, jE, pair, st = info
            ch = slice(qc * 512, (qc + 1) * 512)
            jlast = 4 * qc + 3
            # running denominator sum on DVE; diagonal halves only cover
            # [lo:512], so the adds are ranged (nothing reads below lo)
            loE = max(0, jE - 4 * qc) * 128
            loO = max(0, jE + 1 - 4 * qc) * 128
            acc = st["acc"]
            if acc is None:
                acc = st["acc"] = apool.tile([128, 512], BF, tag="acc",
                                             name=f"acc{qc}_{h}")
                if loO == 0:
                    nc.vector.tensor_add(acc[:], pair[:, 0, :],
                                         pair[:, 1, :])
                else:  # qc==0 first pair: half0 full, half1 from 128
                    nc.vector.tensor_copy(acc[:], pair[:, 0, :])
                    nc.vector.tensor_add(acc[:, loO:512], acc[:, loO:512],
                                         pair[:, 1, loO:512])
            else:
                nc.vector.tensor_add(acc[:, loE:512], acc[:, loE:512],
                                     pair[:, 0, loE:512])
                nc.vector.tensor_add(acc[:, loO:512], acc[:, loO:512],
                                     pair[:, 1, loO:512])
            for half, j in ((0, jE), (1, jE + 1)):
                r = j - 4 * qc
                lo = 128 * r if r > 0 else 0
                nc.tensor.matmul(
                    st["ctx"][:, lo:512],
                    v_s[:, j, h * 128:(h + 1) * 128],
                    pair[:, half, lo:512],
                    start=(j == 0), stop=(j == jlast))
            if jE + 1 == jlast:  # head complete: denominator + normalize
                dent = scps.tile([128, 2, 512], F32, tag="sc",
                                 name=f"den{qc}_{h}")
                den = dent[0:1, 0, :]
                nc.tensor.matmul(den, ones_s[:], st["acc"][:],
                                 start=True, stop=True)
                rc = spool.tile([1, 512], F32, tag="recip")
                nc.vector.reciprocal_approx_fast(rc[:], den)
                rb = spool.tile([128, 512], F32, tag="rbc")
                if qc == NCH - 1 and h == HPC - 1:
                    # final head gates outproj(3): normalize in two halves
                    # so the first outproj units start ~1us sooner
                    for p_ in range(2):
                        sl = slice(256 * p_, 256 * p_ + 256)
                        osl = slice(qc * 512 + 256 * p_,
                                    qc * 512 + 256 * p_ + 256)
                        nc.gpsimd.partition_broadcast(rb[:, sl],
                                                      rc[0:1, sl], 128)
                        nc.vector.tensor_mul(ctxT[h][:, osl],
                                             st["ctx"][:, sl], rb[:, sl])
                else:
                    nc.gpsimd.partition_broadcast(rb[:], rc[:], 128)
                    nc.vector.tensor_mul(ctxT[h][:, ch], st["ctx"][:],
                                         rb[:])

        def legal(entry):
            qc, h, jE = entry
            need = (jE + 1) - 4 * qc + 1  # V evacs of chunk qc required
            return need <= 0 or vdone.get(qc, 0) >= need

        def inject_one():
            # lag-2 pipeline: PV trails its scores by two injection slots
            if len(fifo) >= 2 or (fifo and not (pending
                                                and legal(pending[0]))):
                emit_pv_part(fifo.popleft())
            if pending and legal(pending[0]) and len(fifo) < 2:
                fifo.append(emit_scores_part(*pending.popleft()))

        def tick(nmm):
            sched["budget"] += nmm
            # stale chunks (behind the current one) drain with priority
            ratio = sched["ratio"]
            if pending and pending[0][0] < sched["qc"]:
                ratio = 2.0
            if sched["budget"] >= ratio and (
                    fifo or (pending and legal(pending[0]))):
                sched["budget"] -= ratio
                inject_one()

        def drain_chunk(limit):
            # force-emit every att pair of chunks <= limit
            guard = 0
            while ((fifo and fifo[0][0] <= limit)
                   or (pending and pending[0][0] <= limit)):
                guard += 1
                assert guard < 10000, "drain_chunk stuck"
                if (pending and pending[0][0] <= limit
                        and legal(pending[0]) and len(fifo) < 2):
                    fifo.append(emit_scores_part(*pending.popleft()))
                elif fifo:
                    emit_pv_part(fifo.popleft())
                else:
                    raise RuntimeError("drain_chunk: illegal head blocked")

        # ---- chain emitters --------------------------------------------
        def rope(dst, ch):
            tmp = rpool.tile([128, 512], BF, tag="rtmp")
            t1 = rpool.tile([128, 512], BF, tag="rt1")
            nc.vector.stream_shuffle(tmp[:], dst[:, ch], SWAP_MASK)
            nc.vector.tensor_mul(t1[:], dst[:, ch], cos_s[:, ch])
            nc.vector.tensor_mul(tmp[:], tmp[:], sin_s[:, ch])
            nc.vector.tensor_add(dst[:, ch], t1[:], tmp[:])

        def qk_chain(qc, m, w_s, dst, ps=None, interleave=True):
            ch = slice(qc * 512, (qc + 1) * 512)
            xt = xt_tiles[qc]
            own = ps is None
            if own:
                ps = floatps.tile([128, 512], F32, tag="float",
                                  name=f"qk{qc}_{m}")
            for d in range(ND):
                nc.tensor.matmul(ps[:], w_s[:, d, m * 128:(m + 1) * 128],
                                 xt[:, d, :], start=(d == 0),
                                 stop=(d == ND - 1))
                if interleave and d % 2 == 1:
                    tick(2)
            nc.scalar.copy(dst[:, ch], ps[:])
            rope(dst, ch)

        def v_chain(qc, tt):
            xt = xt_tiles[qc]
            ps = floatps.tile([128, 512], F32, tag="float",
                              name=f"v{qc}_{tt}")
            for d in range(ND):
                nc.tensor.matmul(ps[:], xt[:, d, tt * 128:(tt + 1) * 128],
                                 wv_s[:, d, :], start=(d == 0),
                                 stop=(d == ND - 1))
                if d % 2 == 1:
                    tick(2)
            nc.vector.tensor_copy(v_s[:, qc * 4 + tt, :], ps[:])
            vdone[qc] = vdone.get(qc, 0) + 1

        osb_tiles = {}

        def outproj_unit(qc, tt, dc):
            # tt is the global 128-row token tile index
            if dc % 2 == 0:
                osb_tiles[tt] = outpool.tile([128, 1024], BF, tag="osb",
                                             name=f"osb{tt}_{dc}")
            ps = floatps.tile([128, 512], F32, tag="float",
                              name=f"op{tt}_{dc}")
            for e in range(HPC):
                nc.tensor.matmul(
                    ps[:], ctxT[e][:, tt * 128:(tt + 1) * 128],
                    wo_s[:, e, dc * 512:(dc + 1) * 512],
                    start=(e == 0), stop=(e == HPC - 1))
                if e % 2 == 1:
                    tick(2)
            nc.scalar.copy(
                osb_tiles[tt][:, (dc % 2) * 512:(dc % 2) * 512 + 512],
                ps[:])
            eng = nc.gpsimd if tt % 2 == 0 else nc.sync
            if qc == NCH - 1:
                # tail: store each 512-wide piece as soon as it lands
                eng.dma_start(
                    out[tt * 128:(tt + 1) * 128,
                        dc * 512:(dc + 1) * 512],
                    osb_tiles[tt][:, (dc % 2) * 512:(dc % 2) * 512 + 512])
            elif dc % 2 == 1:
                eng.dma_start(
                    out[tt * 128:(tt + 1) * 128,
                        (dc - 1) * 512:(dc + 1) * 512],
                    osb_tiles[tt][:])

        # ---- the pipelined stream --------------------------------------
        for qc in range(NCH):
            sched["qc"] = qc
            # prefetch x for chunk qc+1 (qc>=1; chunks 0/1 issued at t=0)
            if qc >= 1 and qc + 1 < NCH:
                emit_xt_dma(qc + 1)
            # section 1: QK projections + rope, att(qc-1) injected
            if qc == 0:
                # phase A: 4 q-chains interleaved at d-group granularity
                bootq = [floatps.tile([128, 512], F32, tag="float",
                                      name=f"bootq{m}") for m in range(2)]
                scb = scps.tile([128, 2, 512], F32, tag="sc", name="scboot")
                psA = [t[:] for t in bootq] + [scb[:, 0, :], scb[:, 1, :]]
                for g in range(4):
                    for m in range(4):
                        for dd in range(4):
                            d = g * 4 + dd
                            nc.tensor.matmul(
                                psA[m],
                                wq_s[:, d, m * 128:(m + 1) * 128],
                                xt_tiles[0][:, d, :],
                                start=(d == 0), stop=(d == ND - 1))
                for m in range(4):
                    nc.scalar.copy(qt[m][:, 0:512], psA[m])
                    rope(qt[m], slice(0, 512))
                # phase B: 4 k-chains
                emit_xt_dma(1)
                scb2 = scps.tile([128, 2, 512], F32, tag="sc",
                                 name="scboot2")
                bootk1 = ctxps.tile([128, 512], F32, tag="ctx",
                                    name="bootk1")
                bootk2 = ctxps.tile([128, 512], F32, tag="ctx",
                                    name="bootk2")
                psB = [scb2[:, 0, :], scb2[:, 1, :], bootk1[:], bootk2[:]]
                for g in range(4):
                    for m in range(4):
                        for dd in range(4):
                            d = g * 4 + dd
                            nc.tensor.matmul(
                                psB[m], wk_s[:, d, m * 128:(m + 1) * 128],
                                xt_tiles[0][:, d, :],
                                start=(d == 0), stop=(d == ND - 1))
                for m in range(4):
                    nc.scalar.copy(kt[m][:, 0:512], psB[m])
                    rope(kt[m], slice(0, 512))
            else:
                # QK + V chains per head; push that head's att(qc) pairs as
                # soon as its rope is emitted so they spread over the whole
                # chunk's ~256 floater matmuls
                for m in range(HPC):
                    qk_chain(qc, m, wq_s, qt[m])
                    qk_chain(qc, m, wk_s, kt[m])
                    v_chain(qc, m)
                    for jE in range(0, 4 * qc + 4, 2):
                        pending.append((qc, m, jE))
            # section 2: outproj(qc-1) (V(0) chains for qc=0)
            if qc == 0:
                for h in range(HPC):
                    for jE in range(0, 4, 2):
                        pending.append((qc, h, jE))
                for e in range(HPC):
                    nc.gpsimd.dma_start(wo_s[:, e, :],
                                        woT[e * 128:(e + 1) * 128, :])
                for tt in range(4):
                    v_chain(qc, tt)
            else:
                drain_chunk(qc - 1)  # att(qc-1) done before outproj(qc-1)
                # pre-inject a few att(qc) pairs so the PE has work while
                # the last head's denominator/normalize chain resolves
                for _ in range(4):
                    if fifo or (pending and legal(pending[0])):
                        inject_one()
                for i in range(4):
                    tt = 4 * (qc - 1) + i
                    for dc in range(4):
                        outproj_unit(qc - 1, tt, dc)
        # tail: att(3) leftovers + outproj(3)
        drain_chunk(3)
        for i in range(4):
            tt = 12 + i
            for dc in range(4):
                outproj_unit(3, tt, dc)


def _host_prep(x, freqs_cos, freqs_sin, mask, wq, wk, wv, wo):
    """Build per-core input dicts with pre-grouped DMA-friendly layouts."""
    x = np.asarray(x, np.float32)
    wq = np.asarray(wq, np.float32)
    wk = np.asarray(wk, np.float32)
    wv = np.asarray(wv, np.float32)
    wo = np.asarray(wo, np.float32)
    cos = np.asarray(freqs_cos, np.float32)
    sin = np.asarray(freqs_sin, np.float32)

    # per-head row permutation: quadrant qd holds re rows of pairs
    # 16qd..16qd+15 then their im rows, so the 32-wide stream_shuffle
    # half-swap realises rotate-half
    perm = np.concatenate(
        [np.concatenate([np.arange(32 * qd, 32 * qd + 32, 2),
                         np.arange(32 * qd + 1, 32 * qd + 32, 2)])
         for qd in range(4)])
    p_idx = np.arange(128)
    kmap = 16 * (p_idx // 32) + (p_idx % 16)
    sign = np.where(p_idx % 32 < 16, -1.0, 1.0).astype(np.float32)
    cos2 = np.ascontiguousarray(cos.T[kmap, :]).astype(BF16)
    sin2 = np.ascontiguousarray(sin.T[kmap, :] * sign[:, None]).astype(BF16)
    # boundary triangle: tri[tk, qq] = 1 iff tk <= qq
    tri = np.triu(np.ones((128, 128), np.float32)).astype(BF16)

    def group_w(wT):  # [D, OSL] -> [OSL, 2048]: row g*128+p = tiles 4g..4g+3
        return np.ascontiguousarray(
            wT.reshape(4, 4, 128, OSL).transpose(0, 2, 1, 3)
            .reshape(4 * 128, 4 * OSL))

    def group_x(xT):  # [D, S] -> [S, 2048]: row (qc*4+g)*128+p
        a = xT.reshape(4, 4, 128, 4, 512).transpose(3, 0, 2, 1, 4)
        return np.ascontiguousarray(a.reshape(16 * 128, 2048))

    in_maps = []
    for c in range(NCORE):
        b = c // 4
        o0 = OSL * (c % 4)
        rows = np.concatenate(
            [o0 + h * HD + perm for h in range(HPC)])
        xT = np.ascontiguousarray(x[b].T).astype(BF16)
        in_maps.append(dict(
            xtg=group_x(xT),
            wqg=group_w(np.ascontiguousarray(wq[rows].T).astype(BF16)),
            wkg=group_w(np.ascontiguousarray(wk[rows].T).astype(BF16)),
            wvg=group_w(np.ascontiguousarray(wv[o0:o0 + OSL].T).astype(BF16)),
            woT=np.ascontiguousarray(wo[:, o0:o0 + OSL].T).astype(BF16),
            cos2=cos2, sin2=sin2, tri=tri,
        ))
    return in_maps


_NC_CACHE = None


def get_program():
    global _NC_CACHE
    if _NC_CACHE is None:
        _NC_CACHE = _build_program()
    return _NC_CACHE


def run_on_cores(in_maps, trace=False):
    nc = get_program()
    return bass_utils.run_bass_kernel_spmd(
        nc, in_maps, core_ids=list(range(NCORE)), trace=trace)


def kernel(x, freqs_cos, freqs_sin, mask, wq, wk, wv, wo, start_pos=0,
           **_ignored):
    in_maps = _host_prep(x, freqs_cos, freqs_sin, mask, wq, wk, wv, wo)
    res = run_on_cores(in_maps, trace=False)
    outs = [res.results[c]["out"] for c in range(NCORE)]
    full = np.empty((B, S, D), np.float32)
    for b in range(B):
        acc = outs[4 * b].astype(np.float32)
        for c in range(4 * b + 1, 4 * b + 4):
            acc = acc + outs[c]
        full[b] = acc
    return full
